# revision 1
# baseline (speedup 1.0000x reference)
"""Trainium2 Bass kernel for nn_AttentionGate_22617297781349.

Reference computation (B=128, T=512, D=256):
    z      = concat(facts*q, facts*m, |facts-q|, |facts-m|)   # [B,T,4D]
    g      = tanh(z @ W1 + b1)                                # [B,T,50]
    logits = g @ W2 + b2                                      # [B,T,1]
    out    = softmax(logits, axis=-1)                         # [B,T,1]

The final softmax is over the last axis, which has size 1, so
out[b,t,0] = exp(x - x) / sum(exp(x - x)) = 1.0 exactly, for every
input. Everything upstream of the softmax is dead code; the
mathematically exact kernel is the constant function ones((B,T,1)).

The kernel is data-parallel over the batch dim per the sharding hint:
core i owns batches [16*i, 16*i+16) and materializes its [16,512,1]
shard of ones on-device (SBUF memset -> DMA to DRAM), the host
concatenates the 8 shards.
"""

import sys

if "/opt/trn_rl_repo" not in sys.path:
    sys.path.insert(0, "/opt/trn_rl_repo")

import numpy as np

B, T, D = 128, 512, 256
N_CORES = 8
B_SHARD = B // N_CORES  # 16 batches per core
# Per-core output: [16, 512, 1] = 8192 f32, laid out [128, 64] in SBUF/DRAM.
OUT_P, OUT_F = 128, (B_SHARD * T) // 128

_CACHE = {}


def _build_module():
    import concourse.bass as bass
    import concourse.mybir as mybir

    nc = bass.Bass()
    out = nc.dram_tensor("out", [OUT_P, OUT_F], mybir.dt.float32, kind="ExternalOutput")

    with (
        nc.sbuf_tensor("ones", [OUT_P, OUT_F], mybir.dt.float32) as t,
        nc.semaphore("dma_sem") as dma_sem,
        nc.Block() as block,
    ):

        @block.gpsimd
        def _(gpsimd):
            gpsimd.memset(t[:], 1.0)
            gpsimd.dma_start(out[:], t[:]).then_inc(dma_sem, 16)
            gpsimd.wait_ge(dma_sem, 16)

    return nc


def _run(trace=False):
    from concourse.bass_utils import run_bass_kernel_spmd

    if "nc" not in _CACHE:
        _CACHE["nc"] = _build_module()
    in_maps = [{} for _ in range(N_CORES)]
    return run_bass_kernel_spmd(_CACHE["nc"], in_maps, list(range(N_CORES)), trace=trace)


def kernel(facts, question, memory, W1, b1, W2, b2):
    res = _run(trace=False)
    shards = [r["out"].reshape(B_SHARD, T, 1) for r in res.results]
    return np.ascontiguousarray(np.concatenate(shards, axis=0), dtype=np.float32)


# revision 2
# speedup vs baseline: 1.2177x; 1.2177x over previous
"""Trainium2 Bass kernel for nn_AttentionGate_22617297781349.

Reference computation (B=128, T=512, D=256):
    z      = concat(facts*q, facts*m, |facts-q|, |facts-m|)   # [B,T,4D]
    g      = tanh(z @ W1 + b1)                                # [B,T,50]
    logits = g @ W2 + b2                                      # [B,T,1]
    out    = softmax(logits, axis=-1)                         # [B,T,1]

The final softmax is over the last axis, which has size 1, so
out[b,t,0] = exp(x - x) / sum(exp(x - x)) = 1.0 exactly, for every
finite input (the reference itself notes "== ones, faithful to
original"). Everything upstream of the softmax is dead code; the
mathematically exact kernel is the constant function ones((B,T,1)).

Data-parallel over the batch dim per the sharding hint: core i owns
batches [16*i, 16*i+16) and materializes its [16,512,1] shard of ones
on-device (GpSimd memset of a [128,64] f32 SBUF tile -> SWDGE DMA to
DRAM -> semaphore wait), and the host concatenates the 8 shards.

The module preamble bass emits by default (four const-tensor memsets,
an all-engine drain+event-semaphore barrier, and per-engine register
initialization) is stripped after tracing: this kernel runs on the
GpSimd engine only, touches none of the const tensors, and has no
cross-engine dependencies, so the barrier and the other engines'
register setup are dead weight (~2us of a ~12us NEFF otherwise).
"""

import sys

if "/opt/trn_rl_repo" not in sys.path:
    sys.path.insert(0, "/opt/trn_rl_repo")

import numpy as np

B, T, D = 128, 512, 256
N_CORES = 8
B_SHARD = B // N_CORES  # 16 batches per core
# Per-core output: [16, 512, 1] = 8192 f32, laid out [128, 64] on device.
OUT_P, OUT_F = 128, (B_SHARD * T) // 128

_CACHE = {}

_STRIP_TYPES = ("InstMemset", "InstDrain", "InstEventSemaphore", "InstRegisterMove")


def _build_module():
    import concourse.bass as bass
    import concourse.mybir as mybir

    nc = bass.Bass()
    # Names of the builtin preamble instructions (emitted inside Bass()):
    # everything emitted from here on is this kernel's.
    preamble = {
        ins.name for bb in nc.main_func.blocks for ins in bb.instructions
    }

    out = nc.dram_tensor("out", [OUT_P, OUT_F], mybir.dt.float32, kind="ExternalOutput")

    with (
        nc.sbuf_tensor("ones", [OUT_P, OUT_F], mybir.dt.float32) as t,
        nc.semaphore("dma_sem") as dma_sem,
    ):
        nc.gpsimd.memset(t[:], 1.0)
        nc.gpsimd.dma_start(out[:], t[:]).then_inc(dma_sem, 16)
        nc.gpsimd.wait_ge(dma_sem, 16)

    # Strip the unused preamble: const memsets, the all-engine barrier,
    # and register init. Only instructions recorded in `preamble` are
    # touched, so the kernel's own memset/DMA/wait always survive.
    for bb in nc.main_func.blocks:
        drop = [
            ins
            for ins in bb.instructions
            if ins.name in preamble and type(ins).__name__ in _STRIP_TYPES
        ]
        for ins in drop:
            bb.instructions.remove(ins)
    return nc


def _run(trace=False):
    from concourse.bass_utils import run_bass_kernel_spmd

    if "nc" not in _CACHE:
        _CACHE["nc"] = _build_module()
    in_maps = [{} for _ in range(N_CORES)]
    return run_bass_kernel_spmd(_CACHE["nc"], in_maps, list(range(N_CORES)), trace=trace)


def kernel(facts, question, memory, W1, b1, W2, b2):
    res = _run(trace=False)
    shards = [np.asarray(r["out"]).reshape(B_SHARD, T, 1) for r in res.results]
    full = np.concatenate(shards, axis=0)
    return np.ascontiguousarray(full, dtype=np.float32)


# revision 4
# speedup vs baseline: 1.2199x; 1.0019x over previous
"""Trainium2 Bass kernel for nn_AttentionGate_22617297781349.

Reference computation (B=128, T=512, D=256):
    z      = concat(facts*q, facts*m, |facts-q|, |facts-m|)   # [B,T,4D]
    g      = tanh(z @ W1 + b1)                                # [B,T,50]
    logits = g @ W2 + b2                                      # [B,T,1]
    out    = softmax(logits, axis=-1)                         # [B,T,1]

The final softmax is over the last axis, which has size 1, so
out[b,t,0] = exp(x - x) / sum(exp(x - x)) = 1.0 exactly, for every
finite input (the reference itself notes "== ones, faithful to
original"). Everything upstream of the softmax is dead code; the
mathematically exact kernel is the constant function ones((B,T,1)).

Data-parallel over the batch dim per the sharding hint: core i owns
batches [16*i, 16*i+16) and materializes its [16,512,1] shard of ones
on-device (GpSimd memset of a [128,64] f32 SBUF tile -> SWDGE DMA to
DRAM -> semaphore wait), and the host concatenates the 8 shards.

The module preamble bass emits by default (four const-tensor memsets,
an all-engine drain+event-semaphore barrier, and per-engine register
initialization) is stripped after tracing: this kernel runs on the
GpSimd engine only, touches none of the const tensors, and has no
cross-engine dependencies, so the barrier and the other engines'
register setup are dead weight (~2us of a ~12us NEFF otherwise).

First call compiles + runs via bass_utils.run_bass_kernel_spmd; later
calls re-execute the same NEFF through a cached jitted shard_map (the
upstream helper rebuilds its jit closure per call, forcing a ~0.3s
recompile each time).
"""

import sys

if "/opt/trn_rl_repo" not in sys.path:
    sys.path.insert(0, "/opt/trn_rl_repo")

import numpy as np

B, T, D = 128, 512, 256
N_CORES = 8
B_SHARD = B // N_CORES  # 16 batches per core
# Per-core output: [16, 512, 1] = 8192 f32, laid out [128, 64] on device.
OUT_P, OUT_F = 128, (B_SHARD * T) // 128

_CACHE = {}

_STRIP_TYPES = ("InstMemset", "InstDrain", "InstEventSemaphore", "InstRegisterMove")


def _build_module():
    import concourse.bass as bass
    import concourse.mybir as mybir

    nc = bass.Bass()
    # Names of the builtin preamble instructions (emitted inside Bass()):
    # everything emitted from here on is this kernel's.
    preamble = {
        ins.name for bb in nc.main_func.blocks for ins in bb.instructions
    }

    out = nc.dram_tensor("out", [OUT_P, OUT_F], mybir.dt.float32, kind="ExternalOutput")

    with (
        nc.sbuf_tensor("ones", [OUT_P, OUT_F], mybir.dt.float32) as t,
        nc.semaphore("dma_sem") as dma_sem,
    ):
        nc.gpsimd.memset(t[:], 1.0)
        nc.gpsimd.dma_start(out[:], t[:]).then_inc(dma_sem, 16)
        nc.gpsimd.wait_ge(dma_sem, 16)

    # Strip the unused preamble: const memsets, the all-engine barrier,
    # and register init. Only instructions recorded in `preamble` are
    # touched, so the kernel's own memset/DMA/wait always survive.
    for bb in nc.main_func.blocks:
        drop = [
            ins
            for ins in bb.instructions
            if ins.name in preamble and type(ins).__name__ in _STRIP_TYPES
        ]
        for ins in drop:
            bb.instructions.remove(ins)
    return nc


def _get_nc():
    if "nc" not in _CACHE:
        _CACHE["nc"] = _build_module()
    return _CACHE["nc"]


def _run(trace=False):
    """Compile (first call) + execute the Bass kernel on cores 0-7."""
    from concourse.bass_utils import run_bass_kernel_spmd

    in_maps = [{} for _ in range(N_CORES)]
    return run_bass_kernel_spmd(_get_nc(), in_maps, list(range(N_CORES)), trace=trace)


def _exec_fast():
    """Execute the (already compiled) NEFF on cores 0-7 via a cached jitted
    shard_map. Mirrors bass2jax.run_bass_via_pjrt for this module's I/O:
    no external inputs, one f32 [OUT_P, OUT_F] output, partition-id bound
    last. Returns the per-core output arrays."""
    if "sharded" not in _CACHE:
        import jax
        from jax.sharding import Mesh, PartitionSpec
        from jax.experimental.shard_map import shard_map
        from concourse import bass2jax

        bass2jax.install_neuronx_cc_hook()
        nc = _get_nc()

        def _body(zero_out):
            outs = bass2jax._bass_exec_p.bind(
                zero_out,
                bass2jax.partition_id_tensor(),
                out_avals=(jax.core.ShapedArray((OUT_P, OUT_F), np.float32),),
                in_names=("out", nc.partition_id_tensor.name),
                out_names=("out",),
                lowering_input_output_aliases=(),
                sim_require_finite=True,
                sim_require_nnan=True,
                nc=nc,
            )
            return tuple(outs)

        devices = jax.devices()[:N_CORES]
        mesh = Mesh(np.asarray(devices), ("core",))
        _CACHE["sharded"] = jax.jit(
            shard_map(
                _body,
                mesh=mesh,
                in_specs=(PartitionSpec("core"),),
                out_specs=(PartitionSpec("core"),),
                check_rep=False,
            ),
            donate_argnums=(0,),
            keep_unused=True,
        )
    zeros = np.zeros((N_CORES * OUT_P, OUT_F), np.float32)
    (out,) = _CACHE["sharded"](zeros)
    return np.asarray(out).reshape(N_CORES, OUT_P, OUT_F)


def kernel(facts, question, memory, W1, b1, W2, b2):
    if "ran_once" in _CACHE:
        try:
            per_core = _exec_fast()
            shards = [per_core[c].reshape(B_SHARD, T, 1) for c in range(N_CORES)]
            full = np.concatenate(shards, axis=0)
            return np.ascontiguousarray(full, dtype=np.float32)
        except Exception:
            _CACHE.pop("sharded", None)  # fall through to the slow path
    # First call (or fast-path failure): compile + run via run_bass_kernel_spmd.
    res = _run(trace=False)
    _CACHE["ran_once"] = True
    shards = [np.asarray(r["out"]).reshape(B_SHARD, T, 1) for r in res.results]
    full = np.concatenate(shards, axis=0)
    return np.ascontiguousarray(full, dtype=np.float32)
